# revision 24
# baseline (speedup 1.0000x reference)
"""Trainium2 Bass kernel for nn_DenseEquivariantIrrep.

The reference module (group-Fourier transform -> per-irrep block matmul over
input channels -> inverse transform -> bias) is linear in x, so the whole
pipeline collapses into a single fused operator W of shape (IN_F*N_SYMM,
OUT_F*N_SYMM) = (1024, 1024) plus a bias that only depends on the output
feature index.  W is tiny and depends only on the small parameter tensors, so
it is precomputed on the host in float64; the device work is a pure
data-parallel (65536, 1024) @ (1024, 1024) matmul, sharded over batch across
8 NeuronCores (8192 rows each).

The tensor engine contracts over the partition axis, so the moving/stationary
operand needs x with the K axis on partitions.  Rather than burning PE cycles
on 128x128 on-chip transposes (measured: +33% tensor-engine time), each
core's shard is handed to the device already transposed, as xT (1024, 8192)
-- the device still reads/writes the full 32+32 MB per core.

Per-core device pipeline, per 1024-row supertile (8 total):
  one 4 MB DMA loads xT slab [128, 8kc, 1024b] -> for each 128-row slice:
  PE matmuls (float32r: full-rate fp22 multiplies, fp32 accumulation;
  stationary = xT chunk [128k, 128b], moving = W chunk [128k, 512n], K
  accumulated over 8 chunks, N split 2x512 across PSUM banks) -> DVE adds
  bias while copying PSUM->SBUF -> one 4 MB DMA stores y slab.
"""

import sys

import numpy as np

sys.path.insert(0, "/opt/trn_rl_repo")

import concourse.mybir as mybir
import concourse.tile as tile
from concourse import bacc
from concourse.bass_utils import run_bass_kernel_spmd

N_CORES = 8
B = 65536
IN_F = 16
OUT_F = 16
N_SYMM = 64
K = IN_F * N_SYMM   # 1024 contraction dim
N = OUT_F * N_SYMM  # 1024 output dim
P = 128
ROWS = B // N_CORES  # 8192 rows per core
KC = K // P          # 8 contraction chunks
SB = 1024            # supertile batch width (one 4 MB DMA each way)
N_SUPER = ROWS // SB  # 8
F32 = mybir.dt.float32
F32R = mybir.dt.float32r


def _build_w(kernel_params, kernel_idx, fwd_mat, inv_mat):
    """Fused linear operator W[(c,g), (f,g')] in float64, cast to fp32."""
    kp = np.asarray(kernel_params, np.float64)
    fwd = np.asarray(fwd_mat, np.float64)
    inv = np.asarray(inv_mat, np.float64)
    kern = np.zeros((OUT_F, IN_F, N_SYMM), np.float64)
    kern[:, :, np.asarray(kernel_idx)] = kp
    kf = kern @ fwd  # (f, c, m)
    # wh[(c, m'), (f, m'')]: the per-irrep block matmul in Fourier space.
    wh = np.zeros((IN_F, N_SYMM, OUT_F, N_SYMM), np.float64)
    for n in range(4):  # 1-dim irreps
        wh[:, n, :, n] = kf[:, :, n].T
    for n in range(15):  # 2-dim irreps: (i,j) x (j,k) -> (i,k)
        base = 4 + 4 * n
        for i in range(2):
            for j in range(2):
                for k_ in range(2):
                    wh[:, base + 2 * i + j, :, base + 2 * i + k_] = (
                        kf[:, :, base + 2 * j + k_].T
                    )
    t = np.tensordot(fwd, wh, axes=(1, 1))  # (g, c, f, m'')
    w4 = np.tensordot(t, inv, axes=(3, 0))  # (g, c, f, g')
    w = w4.transpose(1, 0, 2, 3).reshape(K, N)
    return np.ascontiguousarray(w, dtype=np.float32)


_NC_CACHE = {}


def _build_nc():
    if "nc" in _NC_CACHE:
        return _NC_CACHE["nc"]

    nc = bacc.Bacc(
        "TRN2",
        target_bir_lowering=False,
        debug=False,
        enable_asserts=False,
        num_devices=N_CORES,
    )
    xt_d = nc.dram_tensor("xt", [K, ROWS], F32R, kind="ExternalInput").ap()
    w_d = nc.dram_tensor("w", [K, N], F32R, kind="ExternalInput").ap()
    bias_d = nc.dram_tensor("biasb", [P, N], F32, kind="ExternalInput").ap()
    y_d = nc.dram_tensor("y", [ROWS, N], F32, kind="ExternalOutput").ap()

    with tile.TileContext(nc) as tc:
        with (
            tc.tile_pool(name="const", bufs=1) as cpool,
            tc.tile_pool(name="xs", bufs=2) as xpool,
            tc.tile_pool(name="ys", bufs=4) as ypool,
            tc.tile_pool(name="psy", bufs=4, space="PSUM") as psypool,
        ):
            # Resident constants. W arrives in per-chunk DMAs (on the ACT
            # HWDGE ring) so the first matmuls only wait for their chunk.
            w_sb = cpool.tile([P, KC, N], F32R, tag="w")
            for kc in range(KC):
                for nh in range(2):
                    nc.scalar.dma_start(
                        out=w_sb[:, kc, nh * 512 : (nh + 1) * 512],
                        in_=w_d[kc * P : (kc + 1) * P, nh * 512 : (nh + 1) * 512],
                    )
            bias_sb = cpool.tile([P, N], F32, tag="bias")
            nc.scalar.dma_start(out=bias_sb, in_=bias_d)

            for st in range(N_SUPER):
                b0 = st * SB
                # xT slab: partition = k within chunk, [kc, b] on free axis.
                x_sb = xpool.tile([P, KC, SB], F32R, tag="x", name=f"x_{st}")
                if st == 0:
                    # Finely chunked so the first matmuls start ASAP; h=0
                    # halves of all chunks first (matches MM consumption).
                    for h in range(SB // 512):
                        for kc in range(KC):
                            nc.sync.dma_start(
                                out=x_sb[:, kc, h * 512 : (h + 1) * 512],
                                in_=xt_d[
                                    kc * P : (kc + 1) * P,
                                    b0 + h * 512 : b0 + (h + 1) * 512,
                                ],
                            )
                else:
                    nc.sync.dma_start(
                        out=x_sb,
                        in_=xt_d[:, b0 : b0 + SB].rearrange("(a p) b -> p a b", p=P),
                    )

                def mm_group(bt, ps_y):
                    for kc in range(KC):
                        lhsT = x_sb[:, kc, bt * P : (bt + 1) * P]
                        for nh in range(2):
                            nc.tensor.matmul(
                                ps_y[nh],
                                lhsT,
                                w_sb[:, kc, nh * 512 : (nh + 1) * 512],
                                start=(kc == 0),
                                stop=(kc == KC - 1),
                            )

                def add_and_store(pair, y_sb, ps_pair):
                    for sub in range(2):
                        for nh in range(2):
                            nc.vector.tensor_add(
                                y_sb[:, sub, nh * 512 : (nh + 1) * 512],
                                ps_pair[sub][nh],
                                bias_sb[:, nh * 512 : (nh + 1) * 512],
                            )
                    nc.scalar.dma_start(
                        out=y_d[
                            b0 + pair * 2 * P : b0 + (pair + 1) * 2 * P, :
                        ].rearrange("(a p) n -> p a n", p=P),
                        in_=y_sb,
                    )

                def new_psy(bt):
                    return [
                        psypool.tile(
                            [P, 512], F32, tag=f"psy{nh}", name=f"psy{nh}_{st}_{bt}"
                        )
                        for nh in range(2)
                    ]

                if st == 0:
                    # Wave A: first 4 bt groups run kc-outer so every W
                    # chunk feeds ~1.7us of matmuls the moment it lands --
                    # keeps the PE paced with the W-load stream instead of
                    # stalling the whole first group on all of W.
                    ps_wave = [new_psy(bt) for bt in range(4)]
                    for kc in range(KC):
                        for bt in range(4):
                            lhsT = x_sb[:, kc, bt * P : (bt + 1) * P]
                            for nh in range(2):
                                nc.tensor.matmul(
                                    ps_wave[bt][nh],
                                    lhsT,
                                    w_sb[:, kc, nh * 512 : (nh + 1) * 512],
                                    start=(kc == 0),
                                    stop=(kc == KC - 1),
                                )
                    for pair in range(2):
                        y_sb = ypool.tile(
                            [P, 2, N], F32, tag="y", name=f"y_{st}_{pair}"
                        )
                        add_and_store(
                            pair, y_sb, ps_wave[pair * 2 : pair * 2 + 2]
                        )
                    rest_pairs = range(2, SB // P // 2)
                else:
                    rest_pairs = range(SB // P // 2)

                for pair in rest_pairs:
                    y_sb = ypool.tile([P, 2, N], F32, tag="y", name=f"y_{st}_{pair}")
                    ps_pair = []
                    for sub in range(2):
                        bt = pair * 2 + sub
                        ps_y = new_psy(bt)
                        mm_group(bt, ps_y)
                        ps_pair.append(ps_y)
                        for nh in range(2):
                            nc.vector.tensor_add(
                                y_sb[:, sub, nh * 512 : (nh + 1) * 512],
                                ps_y[nh],
                                bias_sb[:, nh * 512 : (nh + 1) * 512],
                            )
                    nc.scalar.dma_start(
                        out=y_d[
                            b0 + pair * 2 * P : b0 + (pair + 1) * 2 * P, :
                        ].rearrange("(a p) n -> p a n", p=P),
                        in_=y_sb,
                    )

    nc.compile()
    _NC_CACHE["nc"] = nc
    return nc


def _prepare(x, kernel_params, bias, kernel_idx, fwd_mat, inv_mat):
    w = _build_w(kernel_params, kernel_idx, fwd_mat, inv_mat)
    bias_flat = np.repeat(np.asarray(bias, np.float64), N_SYMM).astype(np.float32)
    bias_b = np.ascontiguousarray(np.broadcast_to(bias_flat, (P, N)))

    # Shard over batch and hand each core its slice K-major (transposed).
    x_flat = np.asarray(x, np.float32).reshape(N_CORES, ROWS, K)
    xt_all = np.ascontiguousarray(x_flat.transpose(0, 2, 1))  # (cores, K, ROWS)

    nc = _build_nc()
    in_maps = [
        {"xt": xt_all[i], "w": w, "biasb": bias_b} for i in range(N_CORES)
    ]
    return nc, in_maps


def kernel(x, kernel_params, bias, kernel_idx, fwd_mat, inv_mat):
    nc, in_maps = _prepare(x, kernel_params, bias, kernel_idx, fwd_mat, inv_mat)
    res = run_bass_kernel_spmd(nc, in_maps, core_ids=list(range(N_CORES)))
    y = np.concatenate([res.results[i]["y"] for i in range(N_CORES)], axis=0)
    return np.ascontiguousarray(y.reshape(B, OUT_F, N_SYMM).astype(np.float32))


# revision 25
# speedup vs baseline: 1.0322x; 1.0322x over previous
"""Trainium2 Bass kernel for nn_DenseEquivariantIrrep.

The reference module (group-Fourier transform -> per-irrep block matmul over
input channels -> inverse transform -> bias) is linear in x, so the whole
pipeline collapses into a single fused operator W of shape (IN_F*N_SYMM,
OUT_F*N_SYMM) = (1024, 1024) plus a bias that only depends on the output
feature index.  W is tiny and depends only on the small parameter tensors, so
it is precomputed on the host in float64; the device work is a pure
data-parallel (65536, 1024) @ (1024, 1024) matmul, sharded over batch across
8 NeuronCores (8192 rows each).

The tensor engine contracts over the partition axis, so the moving/stationary
operand needs x with the K axis on partitions.  Rather than burning PE cycles
on 128x128 on-chip transposes (measured: +33% tensor-engine time), each
core's shard is handed to the device already transposed, as xT (1024, 8192)
-- the device still reads/writes the full 32+32 MB per core.

Per-core device pipeline, per 1024-row supertile (8 total):
  one 4 MB DMA loads xT slab [128, 8kc, 1024b] -> for each 128-row slice:
  PE matmuls (float32r: full-rate fp22 multiplies, fp32 accumulation;
  stationary = xT chunk [128k, 128b], moving = W chunk [128k, 512n], K
  accumulated over 8 chunks, N split 2x512 across PSUM banks) -> DVE adds
  bias while copying PSUM->SBUF -> one 4 MB DMA stores y slab.
"""

import sys

import numpy as np

sys.path.insert(0, "/opt/trn_rl_repo")

import concourse.mybir as mybir
import concourse.tile as tile
from concourse import bacc
from concourse.bass_utils import run_bass_kernel_spmd

N_CORES = 8
B = 65536
IN_F = 16
OUT_F = 16
N_SYMM = 64
K = IN_F * N_SYMM   # 1024 contraction dim
N = OUT_F * N_SYMM  # 1024 output dim
P = 128
ROWS = B // N_CORES  # 8192 rows per core
KC = K // P          # 8 contraction chunks
SB = 1024            # supertile batch width (one 4 MB DMA each way)
N_SUPER = ROWS // SB  # 8
F32 = mybir.dt.float32
F32R = mybir.dt.float32r


def _build_w(kernel_params, kernel_idx, fwd_mat, inv_mat):
    """Fused linear operator W[(c,g), (f,g')] in float64, cast to fp32."""
    kp = np.asarray(kernel_params, np.float64)
    fwd = np.asarray(fwd_mat, np.float64)
    inv = np.asarray(inv_mat, np.float64)
    kern = np.zeros((OUT_F, IN_F, N_SYMM), np.float64)
    kern[:, :, np.asarray(kernel_idx)] = kp
    kf = kern @ fwd  # (f, c, m)
    # wh[(c, m'), (f, m'')]: the per-irrep block matmul in Fourier space.
    wh = np.zeros((IN_F, N_SYMM, OUT_F, N_SYMM), np.float64)
    for n in range(4):  # 1-dim irreps
        wh[:, n, :, n] = kf[:, :, n].T
    for n in range(15):  # 2-dim irreps: (i,j) x (j,k) -> (i,k)
        base = 4 + 4 * n
        for i in range(2):
            for j in range(2):
                for k_ in range(2):
                    wh[:, base + 2 * i + j, :, base + 2 * i + k_] = (
                        kf[:, :, base + 2 * j + k_].T
                    )
    t = np.tensordot(fwd, wh, axes=(1, 1))  # (g, c, f, m'')
    w4 = np.tensordot(t, inv, axes=(3, 0))  # (g, c, f, g')
    w = w4.transpose(1, 0, 2, 3).reshape(K, N)
    return np.ascontiguousarray(w, dtype=np.float32)


_NC_CACHE = {}


def _build_nc():
    if "nc" in _NC_CACHE:
        return _NC_CACHE["nc"]

    nc = bacc.Bacc(
        "TRN2",
        target_bir_lowering=False,
        debug=False,
        enable_asserts=False,
        num_devices=N_CORES,
    )
    xt_d = nc.dram_tensor("xt", [K, ROWS], F32R, kind="ExternalInput").ap()
    w_d = nc.dram_tensor("w", [K, N], F32R, kind="ExternalInput").ap()
    bias_d = nc.dram_tensor("biasb", [P, N], F32, kind="ExternalInput").ap()
    y_d = nc.dram_tensor("y", [ROWS, N], F32, kind="ExternalOutput").ap()

    with tile.TileContext(nc) as tc:
        with (
            tc.tile_pool(name="const", bufs=1) as cpool,
            tc.tile_pool(name="xs", bufs=2) as xpool,
            tc.tile_pool(name="ys", bufs=4) as ypool,
            tc.tile_pool(name="psy", bufs=4, space="PSUM") as psypool,
        ):
            # Resident constants. W arrives in per-chunk DMAs (on the ACT
            # HWDGE ring) so the first matmuls only wait for their chunk.
            w_sb = cpool.tile([P, KC, N], F32R, tag="w")
            for kc in range(KC):
                nc.scalar.dma_start(
                    out=w_sb[:, kc], in_=w_d[kc * P : (kc + 1) * P, :]
                )
            bias_sb = cpool.tile([P, N], F32, tag="bias")
            nc.scalar.dma_start(out=bias_sb, in_=bias_d)

            for st in range(N_SUPER):
                b0 = st * SB
                # xT slab: partition = k within chunk, [kc, b] on free axis.
                x_sb = xpool.tile([P, KC, SB], F32R, tag="x", name=f"x_{st}")
                if st == 0:
                    # Finely chunked so the first matmuls start ASAP.
                    for kc in range(KC):
                        for h in range(SB // 512):
                            nc.sync.dma_start(
                                out=x_sb[:, kc, h * 512 : (h + 1) * 512],
                                in_=xt_d[
                                    kc * P : (kc + 1) * P,
                                    b0 + h * 512 : b0 + (h + 1) * 512,
                                ],
                            )
                else:
                    nc.sync.dma_start(
                        out=x_sb,
                        in_=xt_d[:, b0 : b0 + SB].rearrange("(a p) b -> p a b", p=P),
                    )

                for pair in range(SB // P // 2):
                    y_sb = ypool.tile([P, 2, N], F32, tag="y", name=f"y_{st}_{pair}")
                    for sub in range(2):
                        bt = pair * 2 + sub
                        ps_y = [
                            psypool.tile(
                                [P, 512], F32, tag=f"psy{nh}",
                                name=f"psy{nh}_{st}_{bt}",
                            )
                            for nh in range(2)
                        ]
                        for kc in range(KC):
                            lhsT = x_sb[:, kc, bt * P : (bt + 1) * P]
                            for nh in range(2):
                                nc.tensor.matmul(
                                    ps_y[nh],
                                    lhsT,
                                    w_sb[:, kc, nh * 512 : (nh + 1) * 512],
                                    start=(kc == 0),
                                    stop=(kc == KC - 1),
                                )
                        for nh in range(2):
                            nc.vector.tensor_add(
                                y_sb[:, sub, nh * 512 : (nh + 1) * 512],
                                ps_y[nh],
                                bias_sb[:, nh * 512 : (nh + 1) * 512],
                            )
                    nc.scalar.dma_start(
                        out=y_d[
                            b0 + pair * 2 * P : b0 + (pair + 1) * 2 * P, :
                        ].rearrange("(a p) n -> p a n", p=P),
                        in_=y_sb,
                    )

    nc.compile()
    _NC_CACHE["nc"] = nc
    return nc


def _prepare(x, kernel_params, bias, kernel_idx, fwd_mat, inv_mat):
    w = _build_w(kernel_params, kernel_idx, fwd_mat, inv_mat)
    bias_flat = np.repeat(np.asarray(bias, np.float64), N_SYMM).astype(np.float32)
    bias_b = np.ascontiguousarray(np.broadcast_to(bias_flat, (P, N)))

    # Shard over batch and hand each core its slice K-major (transposed).
    x_flat = np.asarray(x, np.float32).reshape(N_CORES, ROWS, K)
    xt_all = np.ascontiguousarray(x_flat.transpose(0, 2, 1))  # (cores, K, ROWS)

    nc = _build_nc()
    in_maps = [
        {"xt": xt_all[i], "w": w, "biasb": bias_b} for i in range(N_CORES)
    ]
    return nc, in_maps


def kernel(x, kernel_params, bias, kernel_idx, fwd_mat, inv_mat):
    nc, in_maps = _prepare(x, kernel_params, bias, kernel_idx, fwd_mat, inv_mat)
    res = run_bass_kernel_spmd(nc, in_maps, core_ids=list(range(N_CORES)))
    y = np.concatenate([res.results[i]["y"] for i in range(N_CORES)], axis=0)
    return np.ascontiguousarray(y.reshape(B, OUT_F, N_SYMM).astype(np.float32))


# revision 28
# speedup vs baseline: 1.3787x; 1.3358x over previous
"""Trainium2 Bass kernel for nn_DenseEquivariantIrrep.

The reference module (group-Fourier transform -> per-irrep block matmul over
input channels -> inverse transform -> bias) is linear in x, so the whole
pipeline collapses into a single fused operator W of shape (IN_F*N_SYMM,
OUT_F*N_SYMM) = (1024, 1024) plus a bias that only depends on the output
feature index.  W is tiny and depends only on the small parameter tensors, so
it is precomputed on the host in float64; the device work is a pure
data-parallel (65536, 1024) @ (1024, 1024) matmul, sharded over batch across
8 NeuronCores (8192 rows each).

The tensor engine contracts over the partition axis, so the moving/stationary
operand needs x with the K axis on partitions.  Rather than burning PE cycles
on 128x128 on-chip transposes (measured: +33% tensor-engine time), each
core's shard is handed to the device already transposed, as xT (1024, 8192)
-- the device still reads/writes the full 32+32 MB per core.

Per-core device pipeline, per 1024-row supertile (8 total):
  one 4 MB DMA loads xT slab [128, 8kc, 1024b] -> for each 128-row slice:
  PE matmuls (float32r: full-rate fp22 multiplies, fp32 accumulation;
  stationary = xT chunk [128k, 128b], moving = W chunk [128k, 512n], K
  accumulated over 8 chunks, N split 2x512 across PSUM banks) -> DVE adds
  bias while copying PSUM->SBUF -> one 4 MB DMA stores y slab.
"""

import sys

import numpy as np

sys.path.insert(0, "/opt/trn_rl_repo")

import concourse.mybir as mybir
import concourse.tile as tile
from concourse import bacc
from concourse.bass_utils import run_bass_kernel_spmd

N_CORES = 8
B = 65536
IN_F = 16
OUT_F = 16
N_SYMM = 64
K = IN_F * N_SYMM   # 1024 contraction dim
N = OUT_F * N_SYMM  # 1024 output dim
P = 128
ROWS = B // N_CORES  # 8192 rows per core
KC = K // P          # 8 contraction chunks
SB = 1024            # supertile batch width (one 4 MB DMA each way)
N_SUPER = ROWS // SB  # 8
F32 = mybir.dt.float32
F32R = mybir.dt.float32r


def _build_w(kernel_params, kernel_idx, fwd_mat, inv_mat):
    """Fused linear operator W[(c,g), (f,g')] in float64, cast to fp32."""
    kp = np.asarray(kernel_params, np.float64)
    fwd = np.asarray(fwd_mat, np.float64)
    inv = np.asarray(inv_mat, np.float64)
    kern = np.zeros((OUT_F, IN_F, N_SYMM), np.float64)
    kern[:, :, np.asarray(kernel_idx)] = kp
    kf = kern @ fwd  # (f, c, m)
    # wh[(c, m'), (f, m'')]: the per-irrep block matmul in Fourier space.
    wh = np.zeros((IN_F, N_SYMM, OUT_F, N_SYMM), np.float64)
    for n in range(4):  # 1-dim irreps
        wh[:, n, :, n] = kf[:, :, n].T
    for n in range(15):  # 2-dim irreps: (i,j) x (j,k) -> (i,k)
        base = 4 + 4 * n
        for i in range(2):
            for j in range(2):
                for k_ in range(2):
                    wh[:, base + 2 * i + j, :, base + 2 * i + k_] = (
                        kf[:, :, base + 2 * j + k_].T
                    )
    t = np.tensordot(fwd, wh, axes=(1, 1))  # (g, c, f, m'')
    w4 = np.tensordot(t, inv, axes=(3, 0))  # (g, c, f, g')
    w = w4.transpose(1, 0, 2, 3).reshape(K, N)
    return np.ascontiguousarray(w, dtype=np.float32)


_NC_CACHE = {}


def _build_nc_dense():
    if "dense" in _NC_CACHE:
        return _NC_CACHE["dense"]

    nc = bacc.Bacc(
        "TRN2",
        target_bir_lowering=False,
        debug=False,
        enable_asserts=False,
        num_devices=N_CORES,
    )
    xt_d = nc.dram_tensor("xt", [K, ROWS], F32R, kind="ExternalInput").ap()
    w_d = nc.dram_tensor("w", [K, N], F32R, kind="ExternalInput").ap()
    bias_d = nc.dram_tensor("biasb", [P, N], F32, kind="ExternalInput").ap()
    y_d = nc.dram_tensor("y", [ROWS, N], F32, kind="ExternalOutput").ap()

    with tile.TileContext(nc) as tc:
        with (
            tc.tile_pool(name="const", bufs=1) as cpool,
            tc.tile_pool(name="xs", bufs=2) as xpool,
            tc.tile_pool(name="ys", bufs=4) as ypool,
            tc.tile_pool(name="psy", bufs=4, space="PSUM") as psypool,
        ):
            # Resident constants. W arrives in per-chunk DMAs (on the ACT
            # HWDGE ring) so the first matmuls only wait for their chunk.
            w_sb = cpool.tile([P, KC, N], F32R, tag="w")
            for kc in range(KC):
                nc.scalar.dma_start(
                    out=w_sb[:, kc], in_=w_d[kc * P : (kc + 1) * P, :]
                )
            bias_sb = cpool.tile([P, N], F32, tag="bias")
            nc.scalar.dma_start(out=bias_sb, in_=bias_d)

            for st in range(N_SUPER):
                b0 = st * SB
                # xT slab: partition = k within chunk, [kc, b] on free axis.
                x_sb = xpool.tile([P, KC, SB], F32R, tag="x", name=f"x_{st}")
                if st == 0:
                    # Finely chunked so the first matmuls start ASAP.
                    for kc in range(KC):
                        for h in range(SB // 512):
                            nc.sync.dma_start(
                                out=x_sb[:, kc, h * 512 : (h + 1) * 512],
                                in_=xt_d[
                                    kc * P : (kc + 1) * P,
                                    b0 + h * 512 : b0 + (h + 1) * 512,
                                ],
                            )
                else:
                    nc.sync.dma_start(
                        out=x_sb,
                        in_=xt_d[:, b0 : b0 + SB].rearrange("(a p) b -> p a b", p=P),
                    )

                for pair in range(SB // P // 2):
                    y_sb = ypool.tile([P, 2, N], F32, tag="y", name=f"y_{st}_{pair}")
                    for sub in range(2):
                        bt = pair * 2 + sub
                        ps_y = [
                            psypool.tile(
                                [P, 512], F32, tag=f"psy{nh}",
                                name=f"psy{nh}_{st}_{bt}",
                            )
                            for nh in range(2)
                        ]
                        for kc in range(KC):
                            lhsT = x_sb[:, kc, bt * P : (bt + 1) * P]
                            for nh in range(2):
                                nc.tensor.matmul(
                                    ps_y[nh],
                                    lhsT,
                                    w_sb[:, kc, nh * 512 : (nh + 1) * 512],
                                    start=(kc == 0),
                                    stop=(kc == KC - 1),
                                )
                        for nh in range(2):
                            nc.vector.tensor_add(
                                y_sb[:, sub, nh * 512 : (nh + 1) * 512],
                                ps_y[nh],
                                bias_sb[:, nh * 512 : (nh + 1) * 512],
                            )
                    nc.scalar.dma_start(
                        out=y_d[
                            b0 + pair * 2 * P : b0 + (pair + 1) * 2 * P, :
                        ].rearrange("(a p) n -> p a n", p=P),
                        in_=y_sb,
                    )

    nc.compile()
    _NC_CACHE["dense"] = nc
    return nc


def _build_nc_parity():
    """Half-K variant: the reference kernel is supported on the even group
    elements, an index-2 subgroup (D_16 in D_32), so group convolution never
    mixes the even and odd cosets of the group axis: under an even/odd
    permutation of g, W is two independent 512x512 blocks (cross blocks are
    numerically zero).  K halves, so PE streaming and W traffic halve."""
    if "parity" in _NC_CACHE:
        return _NC_CACHE["parity"]

    KH = KC // 2  # 4 K-chunks per parity
    nc = bacc.Bacc(
        "TRN2",
        target_bir_lowering=False,
        debug=False,
        enable_asserts=False,
        num_devices=N_CORES,
    )
    # xt rows are coset-permuted on the host: rows 0-511 = (c, t) for g=2t,
    # rows 512-1023 = (c, t) for g=2t+1.  w rows follow the same order;
    # w[:512] = W_ee, w[512:] = W_oo, each mapping to 512 output columns
    # (f, u) that the DVE scatters back to natural n = f*64 + 2u + parity.
    xt_d = nc.dram_tensor("xt", [K, ROWS], F32R, kind="ExternalInput").ap()
    w_d = nc.dram_tensor("w", [K, 512], F32R, kind="ExternalInput").ap()
    bias_d = nc.dram_tensor("biasb", [P, 512], F32, kind="ExternalInput").ap()
    y_d = nc.dram_tensor("y", [ROWS, N], F32, kind="ExternalOutput").ap()

    with tile.TileContext(nc) as tc:
        with (
            tc.tile_pool(name="const", bufs=1) as cpool,
            tc.tile_pool(name="xs", bufs=2) as xpool,
            tc.tile_pool(name="ys", bufs=4) as ypool,
            tc.tile_pool(name="psy", bufs=4, space="PSUM") as psypool,
        ):
            w_sb = cpool.tile([P, KC, 512], F32R, tag="w")
            for kc in range(KC):
                nc.scalar.dma_start(
                    out=w_sb[:, kc], in_=w_d[kc * P : (kc + 1) * P, :]
                )
            bias_sb = cpool.tile([P, 512], F32, tag="bias")
            nc.scalar.dma_start(out=bias_sb, in_=bias_d)
            bias_ft = bias_sb.rearrange("p (f t) -> p f t", f=OUT_F)

            for st in range(N_SUPER):
                b0 = st * SB
                x_sb = xpool.tile([P, KC, SB], F32R, tag="x", name=f"x_{st}")
                if st == 0:
                    for kc in range(KC):
                        for h in range(SB // 512):
                            nc.sync.dma_start(
                                out=x_sb[:, kc, h * 512 : (h + 1) * 512],
                                in_=xt_d[
                                    kc * P : (kc + 1) * P,
                                    b0 + h * 512 : b0 + (h + 1) * 512,
                                ],
                            )
                else:
                    nc.sync.dma_start(
                        out=x_sb,
                        in_=xt_d[:, b0 : b0 + SB].rearrange("(a p) b -> p a b", p=P),
                    )

                for pair in range(SB // P // 2):
                    y_sb = ypool.tile([P, 2, N], F32, tag="y", name=f"y_{st}_{pair}")
                    for sub in range(2):
                        bt = pair * 2 + sub
                        for par in range(2):
                            ps_y = psypool.tile(
                                [P, 512], F32, tag=f"psy{par}",
                                name=f"psy{par}_{st}_{bt}",
                            )
                            for kcl in range(KH):
                                kc = par * KH + kcl
                                nc.tensor.matmul(
                                    ps_y,
                                    x_sb[:, kc, bt * P : (bt + 1) * P],
                                    w_sb[:, kc],
                                    start=(kcl == 0),
                                    stop=(kcl == KH - 1),
                                )
                            # scatter (f, u) -> n = f*64 + 2u + par
                            out_ap = y_sb[:, sub].rearrange(
                                "p (f t two) -> p f t two", f=OUT_F, two=2
                            )[:, :, :, par]
                            nc.vector.tensor_add(
                                out_ap,
                                ps_y.rearrange("p (f t) -> p f t", f=OUT_F),
                                bias_ft,
                            )
                    nc.scalar.dma_start(
                        out=y_d[
                            b0 + pair * 2 * P : b0 + (pair + 1) * 2 * P, :
                        ].rearrange("(a p) n -> p a n", p=P),
                        in_=y_sb,
                    )

    nc.compile()
    _NC_CACHE["parity"] = nc
    return nc


_COSET_PERM = np.concatenate(
    [
        (np.arange(IN_F)[:, None] * N_SYMM + 2 * np.arange(32)[None, :]).ravel(),
        (np.arange(IN_F)[:, None] * N_SYMM + 2 * np.arange(32)[None, :] + 1).ravel(),
    ]
)


def _prepare(x, kernel_params, bias, kernel_idx, fwd_mat, inv_mat):
    w = _build_w(kernel_params, kernel_idx, fwd_mat, inv_mat)

    # Coset split: valid iff W has no even<->odd coupling on the group axis
    # (always true for the reference's even-element kernel mask).
    w4 = w.reshape(IN_F, N_SYMM, OUT_F, N_SYMM)
    ev, od = np.arange(0, N_SYMM, 2), np.arange(1, N_SYMM, 2)
    cross = max(
        np.abs(w4[:, ev][:, :, :, od]).max(),
        np.abs(w4[:, od][:, :, :, ev]).max(),
    )
    parity_ok = cross <= 1e-6 * max(np.abs(w).max(), 1e-30)

    if parity_ok:
        w_ee = w4[:, ev][:, :, :, ev].reshape(512, 512)
        w_oo = w4[:, od][:, :, :, od].reshape(512, 512)
        w_packed = np.ascontiguousarray(np.concatenate([w_ee, w_oo], axis=0))
        bias_flat = np.repeat(np.asarray(bias, np.float64), 32).astype(np.float32)
        bias_b = np.ascontiguousarray(np.broadcast_to(bias_flat, (P, 512)))
        x_flat = np.asarray(x, np.float32).reshape(N_CORES, ROWS, K)
        xt_all = np.ascontiguousarray(
            x_flat.transpose(0, 2, 1)[:, _COSET_PERM, :]
        )
        nc = _build_nc_parity()
        in_maps = [
            {"xt": xt_all[i], "w": w_packed, "biasb": bias_b}
            for i in range(N_CORES)
        ]
        return nc, in_maps

    bias_flat = np.repeat(np.asarray(bias, np.float64), N_SYMM).astype(np.float32)
    bias_b = np.ascontiguousarray(np.broadcast_to(bias_flat, (P, N)))

    # Shard over batch and hand each core its slice K-major (transposed).
    x_flat = np.asarray(x, np.float32).reshape(N_CORES, ROWS, K)
    xt_all = np.ascontiguousarray(x_flat.transpose(0, 2, 1))  # (cores, K, ROWS)

    nc = _build_nc_dense()
    in_maps = [
        {"xt": xt_all[i], "w": w, "biasb": bias_b} for i in range(N_CORES)
    ]
    return nc, in_maps


def kernel(x, kernel_params, bias, kernel_idx, fwd_mat, inv_mat):
    nc, in_maps = _prepare(x, kernel_params, bias, kernel_idx, fwd_mat, inv_mat)
    res = run_bass_kernel_spmd(nc, in_maps, core_ids=list(range(N_CORES)))
    y = np.concatenate([res.results[i]["y"] for i in range(N_CORES)], axis=0)
    return np.ascontiguousarray(y.reshape(B, OUT_F, N_SYMM).astype(np.float32))


# revision 29
# speedup vs baseline: 1.4198x; 1.0298x over previous
"""Trainium2 Bass kernel for nn_DenseEquivariantIrrep.

The reference module (group-Fourier transform -> per-irrep block matmul over
input channels -> inverse transform -> bias) is linear in x, so the whole
pipeline collapses into a single fused operator W of shape (IN_F*N_SYMM,
OUT_F*N_SYMM) = (1024, 1024) plus a bias that only depends on the output
feature index.  W is tiny and depends only on the small parameter tensors, so
it is precomputed on the host in float64; the device work is a pure
data-parallel (65536, 1024) @ (1024, 1024) matmul, sharded over batch across
8 NeuronCores (8192 rows each).

The tensor engine contracts over the partition axis, so the moving/stationary
operand needs x with the K axis on partitions.  Rather than burning PE cycles
on 128x128 on-chip transposes (measured: +33% tensor-engine time), each
core's shard is handed to the device already transposed, as xT (1024, 8192)
-- the device still reads/writes the full 32+32 MB per core.

Per-core device pipeline, per 1024-row supertile (8 total):
  one 4 MB DMA loads xT slab [128, 8kc, 1024b] -> for each 128-row slice:
  PE matmuls (float32r: full-rate fp22 multiplies, fp32 accumulation;
  stationary = xT chunk [128k, 128b], moving = W chunk [128k, 512n], K
  accumulated over 8 chunks, N split 2x512 across PSUM banks) -> DVE adds
  bias while copying PSUM->SBUF -> one 4 MB DMA stores y slab.
"""

import sys

import numpy as np

sys.path.insert(0, "/opt/trn_rl_repo")

import concourse.mybir as mybir
import concourse.tile as tile
from concourse import bacc
from concourse.bass_utils import run_bass_kernel_spmd

N_CORES = 8
B = 65536
IN_F = 16
OUT_F = 16
N_SYMM = 64
K = IN_F * N_SYMM   # 1024 contraction dim
N = OUT_F * N_SYMM  # 1024 output dim
P = 128
ROWS = B // N_CORES  # 8192 rows per core
KC = K // P          # 8 contraction chunks
SB = 1024            # supertile batch width (one 4 MB DMA each way)
N_SUPER = ROWS // SB  # 8
F32 = mybir.dt.float32
F32R = mybir.dt.float32r


def _build_w(kernel_params, kernel_idx, fwd_mat, inv_mat):
    """Fused linear operator W[(c,g), (f,g')] in float64, cast to fp32."""
    kp = np.asarray(kernel_params, np.float64)
    fwd = np.asarray(fwd_mat, np.float64)
    inv = np.asarray(inv_mat, np.float64)
    kern = np.zeros((OUT_F, IN_F, N_SYMM), np.float64)
    kern[:, :, np.asarray(kernel_idx)] = kp
    kf = kern @ fwd  # (f, c, m)
    # wh[(c, m'), (f, m'')]: the per-irrep block matmul in Fourier space.
    wh = np.zeros((IN_F, N_SYMM, OUT_F, N_SYMM), np.float64)
    for n in range(4):  # 1-dim irreps
        wh[:, n, :, n] = kf[:, :, n].T
    for n in range(15):  # 2-dim irreps: (i,j) x (j,k) -> (i,k)
        base = 4 + 4 * n
        for i in range(2):
            for j in range(2):
                for k_ in range(2):
                    wh[:, base + 2 * i + j, :, base + 2 * i + k_] = (
                        kf[:, :, base + 2 * j + k_].T
                    )
    t = np.tensordot(fwd, wh, axes=(1, 1))  # (g, c, f, m'')
    w4 = np.tensordot(t, inv, axes=(3, 0))  # (g, c, f, g')
    w = w4.transpose(1, 0, 2, 3).reshape(K, N)
    return np.ascontiguousarray(w, dtype=np.float32)


_NC_CACHE = {}


def _build_nc_dense():
    if "dense" in _NC_CACHE:
        return _NC_CACHE["dense"]

    nc = bacc.Bacc(
        "TRN2",
        target_bir_lowering=False,
        debug=False,
        enable_asserts=False,
        num_devices=N_CORES,
    )
    xt_d = nc.dram_tensor("xt", [K, ROWS], F32R, kind="ExternalInput").ap()
    w_d = nc.dram_tensor("w", [K, N], F32R, kind="ExternalInput").ap()
    bias_d = nc.dram_tensor("biasb", [P, N], F32, kind="ExternalInput").ap()
    y_d = nc.dram_tensor("y", [ROWS, N], F32, kind="ExternalOutput").ap()

    with tile.TileContext(nc) as tc:
        with (
            tc.tile_pool(name="const", bufs=1) as cpool,
            tc.tile_pool(name="xs", bufs=2) as xpool,
            tc.tile_pool(name="ys", bufs=4) as ypool,
            tc.tile_pool(name="psy", bufs=4, space="PSUM") as psypool,
        ):
            # Resident constants. W arrives in per-chunk DMAs (on the ACT
            # HWDGE ring) so the first matmuls only wait for their chunk.
            w_sb = cpool.tile([P, KC, N], F32R, tag="w")
            for kc in range(KC):
                nc.scalar.dma_start(
                    out=w_sb[:, kc], in_=w_d[kc * P : (kc + 1) * P, :]
                )
            bias_sb = cpool.tile([P, N], F32, tag="bias")
            nc.scalar.dma_start(out=bias_sb, in_=bias_d)

            for st in range(N_SUPER):
                b0 = st * SB
                # xT slab: partition = k within chunk, [kc, b] on free axis.
                x_sb = xpool.tile([P, KC, SB], F32R, tag="x", name=f"x_{st}")
                if st == 0:
                    # Finely chunked so the first matmuls start ASAP.
                    for kc in range(KC):
                        for h in range(SB // 512):
                            nc.sync.dma_start(
                                out=x_sb[:, kc, h * 512 : (h + 1) * 512],
                                in_=xt_d[
                                    kc * P : (kc + 1) * P,
                                    b0 + h * 512 : b0 + (h + 1) * 512,
                                ],
                            )
                else:
                    nc.sync.dma_start(
                        out=x_sb,
                        in_=xt_d[:, b0 : b0 + SB].rearrange("(a p) b -> p a b", p=P),
                    )

                for pair in range(SB // P // 2):
                    y_sb = ypool.tile([P, 2, N], F32, tag="y", name=f"y_{st}_{pair}")
                    for sub in range(2):
                        bt = pair * 2 + sub
                        ps_y = [
                            psypool.tile(
                                [P, 512], F32, tag=f"psy{nh}",
                                name=f"psy{nh}_{st}_{bt}",
                            )
                            for nh in range(2)
                        ]
                        for kc in range(KC):
                            lhsT = x_sb[:, kc, bt * P : (bt + 1) * P]
                            for nh in range(2):
                                nc.tensor.matmul(
                                    ps_y[nh],
                                    lhsT,
                                    w_sb[:, kc, nh * 512 : (nh + 1) * 512],
                                    start=(kc == 0),
                                    stop=(kc == KC - 1),
                                )
                        for nh in range(2):
                            nc.vector.tensor_add(
                                y_sb[:, sub, nh * 512 : (nh + 1) * 512],
                                ps_y[nh],
                                bias_sb[:, nh * 512 : (nh + 1) * 512],
                            )
                    nc.scalar.dma_start(
                        out=y_d[
                            b0 + pair * 2 * P : b0 + (pair + 1) * 2 * P, :
                        ].rearrange("(a p) n -> p a n", p=P),
                        in_=y_sb,
                    )

    nc.compile()
    _NC_CACHE["dense"] = nc
    return nc


def _build_nc_parity():
    """Half-K variant: the reference kernel is supported on the even group
    elements, an index-2 subgroup (D_16 in D_32), so group convolution never
    mixes the even and odd cosets of the group axis: under an even/odd
    permutation of g, W is two independent 512x512 blocks (cross blocks are
    numerically zero).  K halves, so PE streaming and W traffic halve."""
    if "parity" in _NC_CACHE:
        return _NC_CACHE["parity"]

    KH = KC // 2  # 4 K-chunks per parity
    SBP = 2048    # wider slabs: 8 KB contiguous runs on the x read stream
    NSP = ROWS // SBP
    nc = bacc.Bacc(
        "TRN2",
        target_bir_lowering=False,
        debug=False,
        enable_asserts=False,
        num_devices=N_CORES,
    )
    # xt rows are coset-permuted on the host: rows 0-511 = (c, t) for g=2t,
    # rows 512-1023 = (c, t) for g=2t+1.  w rows follow the same order;
    # w[:512] = W_ee, w[512:] = W_oo, each mapping to 512 output columns
    # (f, u) that the DVE scatters back to natural n = f*64 + 2u + parity.
    xt_d = nc.dram_tensor("xt", [K, ROWS], F32R, kind="ExternalInput").ap()
    w_d = nc.dram_tensor("w", [K, 512], F32R, kind="ExternalInput").ap()
    bias_d = nc.dram_tensor("biasb", [P, 512], F32, kind="ExternalInput").ap()
    y_d = nc.dram_tensor("y", [ROWS, N], F32, kind="ExternalOutput").ap()

    with tile.TileContext(nc) as tc:
        with (
            tc.tile_pool(name="const", bufs=1) as cpool,
            tc.tile_pool(name="xs", bufs=2) as xpool,
            tc.tile_pool(name="ys", bufs=4) as ypool,
            tc.tile_pool(name="psy", bufs=4, space="PSUM") as psypool,
        ):
            w_sb = cpool.tile([P, KC, 512], F32R, tag="w")
            for kc in range(KC):
                nc.scalar.dma_start(
                    out=w_sb[:, kc], in_=w_d[kc * P : (kc + 1) * P, :]
                )
            bias_sb = cpool.tile([P, 512], F32, tag="bias")
            nc.scalar.dma_start(out=bias_sb, in_=bias_d)
            bias_ft = bias_sb.rearrange("p (f t) -> p f t", f=OUT_F)

            for st in range(NSP):
                b0 = st * SBP
                x_sb = xpool.tile([P, KC, SBP], F32R, tag="x", name=f"x_{st}")
                if st == 0:
                    for kc in range(KC):
                        for h in range(SBP // 512):
                            nc.sync.dma_start(
                                out=x_sb[:, kc, h * 512 : (h + 1) * 512],
                                in_=xt_d[
                                    kc * P : (kc + 1) * P,
                                    b0 + h * 512 : b0 + (h + 1) * 512,
                                ],
                            )
                else:
                    nc.sync.dma_start(
                        out=x_sb,
                        in_=xt_d[:, b0 : b0 + SBP].rearrange("(a p) b -> p a b", p=P),
                    )

                for pair in range(SBP // P // 2):
                    y_sb = ypool.tile([P, 2, N], F32, tag="y", name=f"y_{st}_{pair}")
                    for sub in range(2):
                        bt = pair * 2 + sub
                        for par in range(2):
                            ps_y = psypool.tile(
                                [P, 512], F32, tag=f"psy{par}",
                                name=f"psy{par}_{st}_{bt}",
                            )
                            for kcl in range(KH):
                                kc = par * KH + kcl
                                nc.tensor.matmul(
                                    ps_y,
                                    x_sb[:, kc, bt * P : (bt + 1) * P],
                                    w_sb[:, kc],
                                    start=(kcl == 0),
                                    stop=(kcl == KH - 1),
                                )
                            # scatter (f, u) -> n = f*64 + 2u + par
                            out_ap = y_sb[:, sub].rearrange(
                                "p (f t two) -> p f t two", f=OUT_F, two=2
                            )[:, :, :, par]
                            nc.vector.tensor_add(
                                out_ap,
                                ps_y.rearrange("p (f t) -> p f t", f=OUT_F),
                                bias_ft,
                            )
                    nc.scalar.dma_start(
                        out=y_d[
                            b0 + pair * 2 * P : b0 + (pair + 1) * 2 * P, :
                        ].rearrange("(a p) n -> p a n", p=P),
                        in_=y_sb,
                    )

    nc.compile()
    _NC_CACHE["parity"] = nc
    return nc


_COSET_PERM = np.concatenate(
    [
        (np.arange(IN_F)[:, None] * N_SYMM + 2 * np.arange(32)[None, :]).ravel(),
        (np.arange(IN_F)[:, None] * N_SYMM + 2 * np.arange(32)[None, :] + 1).ravel(),
    ]
)


def _prepare(x, kernel_params, bias, kernel_idx, fwd_mat, inv_mat):
    w = _build_w(kernel_params, kernel_idx, fwd_mat, inv_mat)

    # Coset split: valid iff W has no even<->odd coupling on the group axis
    # (always true for the reference's even-element kernel mask).
    w4 = w.reshape(IN_F, N_SYMM, OUT_F, N_SYMM)
    ev, od = np.arange(0, N_SYMM, 2), np.arange(1, N_SYMM, 2)
    cross = max(
        np.abs(w4[:, ev][:, :, :, od]).max(),
        np.abs(w4[:, od][:, :, :, ev]).max(),
    )
    parity_ok = cross <= 1e-6 * max(np.abs(w).max(), 1e-30)

    if parity_ok:
        w_ee = w4[:, ev][:, :, :, ev].reshape(512, 512)
        w_oo = w4[:, od][:, :, :, od].reshape(512, 512)
        w_packed = np.ascontiguousarray(np.concatenate([w_ee, w_oo], axis=0))
        bias_flat = np.repeat(np.asarray(bias, np.float64), 32).astype(np.float32)
        bias_b = np.ascontiguousarray(np.broadcast_to(bias_flat, (P, 512)))
        x_flat = np.asarray(x, np.float32).reshape(N_CORES, ROWS, K)
        xt_all = np.ascontiguousarray(
            x_flat.transpose(0, 2, 1)[:, _COSET_PERM, :]
        )
        nc = _build_nc_parity()
        in_maps = [
            {"xt": xt_all[i], "w": w_packed, "biasb": bias_b}
            for i in range(N_CORES)
        ]
        return nc, in_maps

    bias_flat = np.repeat(np.asarray(bias, np.float64), N_SYMM).astype(np.float32)
    bias_b = np.ascontiguousarray(np.broadcast_to(bias_flat, (P, N)))

    # Shard over batch and hand each core its slice K-major (transposed).
    x_flat = np.asarray(x, np.float32).reshape(N_CORES, ROWS, K)
    xt_all = np.ascontiguousarray(x_flat.transpose(0, 2, 1))  # (cores, K, ROWS)

    nc = _build_nc_dense()
    in_maps = [
        {"xt": xt_all[i], "w": w, "biasb": bias_b} for i in range(N_CORES)
    ]
    return nc, in_maps


def kernel(x, kernel_params, bias, kernel_idx, fwd_mat, inv_mat):
    nc, in_maps = _prepare(x, kernel_params, bias, kernel_idx, fwd_mat, inv_mat)
    res = run_bass_kernel_spmd(nc, in_maps, core_ids=list(range(N_CORES)))
    y = np.concatenate([res.results[i]["y"] for i in range(N_CORES)], axis=0)
    return np.ascontiguousarray(y.reshape(B, OUT_F, N_SYMM).astype(np.float32))
